# revision 17
# baseline (speedup 1.0000x reference)
"""Trainium2 Bass kernel for nn_CGNN_16827681865778 (gnn_message_passing).

Data-parallel over batch across 8 NeuronCores. Each core processes a
(2048, 20) shard of x and produces its shard of the four outputs.

Per-core dataflow (fp16 matmul operands, fp32 PSUM accumulate/outputs):
  - x is PE-transposed to feature-major xT (20, 512) per 512-batch block.
  - L1 is a circular "conv" folded into packed weights W1big (20, 640):
    neighbor gather happens inside the weights, 5 matmuls of M=128.
  - L2/L3 are block-diagonal packed matmuls over (unit, net) groups.
  - L4 is activation-stationary: lhsT = A3 chunk (33,128) (with a ones row
    carrying the bias), rhs = W4aug (33,146) -> batch-major PSUM (128,146)
    whose columns are ordered [f1 | f2 | g1 band | g2 band].
  - Band values are copied into persistent zero-initialized SBUF slabs
    (zeros are written once; band cells are overwritten every chunk), then
    DMA'd as large contiguous transfers to HBM.
"""

import functools
import os

import numpy as np

B = 16384
NCORES = 8
BC = B // NCORES          # 2048 batch per core
NBLK = BC // 512          # 4 blocks of 512
NT = BC // 128            # 16 chunks of 128

LAST_RESULT = None        # BassKernelResults of the most recent run
INTERLEAVE_T0 = os.environ.get("K_INTERLEAVE", "1") == "1"


def _pack_weights(inp):
    f32 = np.float32
    w1a, w1b = np.asarray(inp["w1a"], f32), np.asarray(inp["w1b"], f32)
    w2a, w2b = np.asarray(inp["w2a"], f32), np.asarray(inp["w2b"], f32)
    w3a, w3b = np.asarray(inp["w3a"], f32), np.asarray(inp["w3b"], f32)
    w4a, w4b = np.asarray(inp["w4a"], f32), np.asarray(inp["w4b"], f32)
    b1a, b1b = np.asarray(inp["b1a"], f32), np.asarray(inp["b1b"], f32)
    b2a, b2b = np.asarray(inp["b2a"], f32), np.asarray(inp["b2b"], f32)
    b3a, b3b = np.asarray(inp["b3a"], f32), np.asarray(inp["b3b"], f32)
    b4a, b4b = np.asarray(inp["b4a"], f32), np.asarray(inp["b4b"], f32)

    W1big = np.zeros((20, 20, 32), f32)
    for u in range(20):
        for d in range(3):
            j = (u - 1 + d) % 20
            W1big[j, u, 0:16] = w1a[d]
            W1big[j, u, 16:32] = w1b[d]
    W1big = np.ascontiguousarray(W1big.reshape(20, 640))

    W2blk = np.zeros((64, 128), f32)
    for ul in range(2):
        W2blk[ul*32+0:ul*32+16,  ul*64+0:ul*64+32] = w2a
        W2blk[ul*32+16:ul*32+32, ul*64+32:ul*64+64] = w2b
    W2stack = np.ascontiguousarray(np.concatenate([W2blk, W2blk], axis=0))

    W3blk = np.zeros((64, 32), f32)
    W3blk[0:32, 0:16] = w3a
    W3blk[32:64, 16:32] = w3b
    W3stack = np.ascontiguousarray(np.concatenate([W3blk, W3blk], axis=0))

    W4aug = np.zeros((33, 146), f32)
    b4 = np.zeros(146, f32)
    W4aug[0:16, 0] = w4a[:, 0];          b4[0] = b4a[0]
    W4aug[16:32, 1:6] = w4b[:, 0:5];     b4[1:6] = b4b[0:5]
    W4aug[0:16, 6:21] = w4a[:, 1:16];    b4[6:21] = b4a[1:16]
    W4aug[16:32, 21:146] = w4b[:, 5:130]; b4[21:146] = b4b[5:130]
    W4aug[32, :] = b4

    b1p = np.zeros((128, 1), f32)
    b2p = np.zeros((128, 1), f32)
    b3p = np.zeros((32, 1), f32)
    for ul in range(4):
        b1p[ul*32+0:ul*32+16, 0] = b1a
        b1p[ul*32+16:ul*32+32, 0] = b1b
    for ul in range(2):
        b2p[ul*64+0:ul*64+32, 0] = b2a
        b2p[ul*64+32:ul*64+64, 0] = b2b
    b3p[0:16, 0] = b3a
    b3p[16:32, 0] = b3b

    f16 = np.float16
    return dict(w1big=W1big.astype(f16), w2stack=W2stack.astype(f16),
                w3stack=W3stack.astype(f16), w4aug=W4aug.astype(f16),
                b1p=b1p, b2p=b2p, b3p=b3p,
                ident=np.eye(128, dtype=f32))


@functools.lru_cache(maxsize=1)
def _build_nc():
    import concourse.bass as bass
    import concourse.bacc as bacc
    import concourse.mybir as mybir
    import concourse.tile as tile

    f32 = mybir.dt.float32
    f16 = mybir.dt.float16
    Relu = mybir.ActivationFunctionType.Relu
    add_op = mybir.AluOpType.add
    max_op = mybir.AluOpType.max

    nc = bacc.Bacc("TRN2", target_bir_lowering=False, debug=False)

    x_d = nc.declare_dram_parameter("x", [BC, 20], f32, isOutput=False)
    w1_d = nc.declare_dram_parameter("w1big", [20, 640], f16, isOutput=False)
    w2_d = nc.declare_dram_parameter("w2stack", [128, 128], f16, isOutput=False)
    w3_d = nc.declare_dram_parameter("w3stack", [128, 32], f16, isOutput=False)
    w4_d = nc.declare_dram_parameter("w4aug", [33, 146], f16, isOutput=False)
    b1_d = nc.declare_dram_parameter("b1p", [128, 1], f32, isOutput=False)
    b2_d = nc.declare_dram_parameter("b2p", [128, 1], f32, isOutput=False)
    b3_d = nc.declare_dram_parameter("b3p", [32, 1], f32, isOutput=False)
    id_d = nc.declare_dram_parameter("ident", [128, 128], f32, isOutput=False)

    f12_d = nc.declare_dram_parameter("f12", [BC, 20, 6], f32, isOutput=True)
    g1_d = nc.declare_dram_parameter("g1", [BC, 20, 100], f32, isOutput=True)
    g2_d = nc.declare_dram_parameter("g2", [BC, 100, 100], f32, isOutput=True)

    # round-robin engine pickers for PSUM->SBUF copies / relus
    state = {"i": 0}

    def copy(out, in_):
        state["i"] += 1
        if state["i"] % 5 in (0, 2, 4):
            nc.vector.tensor_copy(out, in_)
        else:
            nc.scalar.copy(out, in_)

    def relu_bias(out, in_, bias_ap):
        state["i"] += 1
        if state["i"] % 2 == 0:
            nc.vector.tensor_scalar(out, in_, bias_ap, 0.0, add_op, max_op)
        else:
            nc.scalar.activation(out, in_, Relu, bias=bias_ap)

    with tile.TileContext(nc) as tc:
        with (
            tc.tile_pool(name="const", bufs=1) as cpool,
            tc.tile_pool(name="slab", bufs=1) as spool,
            tc.tile_pool(name="act", bufs=2) as apool,
            tc.tile_pool(name="a2p", bufs=3) as a2pool,
            tc.tile_pool(name="xtp", bufs=2) as xtpool,
            tc.tile_pool(name="ps_a", bufs=2, space="PSUM") as ps_a,
            tc.tile_pool(name="ps_3", bufs=2, space="PSUM") as ps_3,
            tc.tile_pool(name="ps_4", bufs=3, space="PSUM") as ps_4,
            tc.tile_pool(name="ps_x", bufs=1, space="PSUM") as ps_x,
        ):
            # constants
            w1_sb = cpool.tile([20, 640], f16, tag="w1")
            w2_sb = cpool.tile([128, 128], f16, tag="w2")
            w3_sb = cpool.tile([128, 32], f16, tag="w3")
            w4_sb = cpool.tile([33, 146], f16, tag="w4")
            b1_sb = cpool.tile([128, 1], f32, tag="b1")
            b2_sb = cpool.tile([128, 1], f32, tag="b2")
            b3_sb = cpool.tile([32, 1], f32, tag="b3")
            id_sb = cpool.tile([128, 128], f32, tag="id")
            x_sb = cpool.tile([128, NT, 20], f32, tag="x")

            nc.sync.dma_start(
                out=x_sb[:], in_=x_d.rearrange("(t p) f -> p t f", p=128)
            )
            nc.sync.dma_start(out=id_sb[:], in_=id_d[:])
            nc.sync.dma_start(out=w1_sb[:], in_=w1_d[:])
            nc.sync.dma_start(out=b1_sb[:], in_=b1_d[:])
            nc.sync.dma_start(out=w2_sb[:], in_=w2_d[:])
            nc.sync.dma_start(out=b2_sb[:], in_=b2_d[:])
            nc.sync.dma_start(out=w3_sb[:], in_=w3_d[:])
            nc.sync.dma_start(out=b3_sb[:], in_=b3_d[:])
            nc.sync.dma_start(out=w4_sb[:], in_=w4_d[:])

            # persistent A3 tiles; row 32 = 1.0 (bias lane), set once
            a3 = [cpool.tile([33, 512], f16, tag=f"a3_{u}", name=f"a3_{u}")
                  for u in range(20)]
            for u in range(10):
                nc.vector.memset(a3[u][32:33, :], 1.0)

            # double-buffered output slabs, zeroed once
            g2_slabs = [[spool.tile([128, 50, 100], f32, tag=f"g2s{i}h{h}",
                                    name=f"g2s{i}h{h}") for h in range(2)]
                        for i in range(2)]
            g1_slabs = [spool.tile([128, 20, 100], f32, tag=f"g1s{i}",
                                   name=f"g1s{i}") for i in range(2)]
            f12_slab = spool.tile([128, NT, 120], f32, tag="f12s")
            # all slab zero-fills on the otherwise-idle gpsimd queue, in
            # need order (buffer 0 first); a3 ones rows u>=10 in between
            nc.gpsimd.memset(g2_slabs[0][0][:], 0.0)
            nc.gpsimd.memset(g2_slabs[0][1][:], 0.0)
            nc.gpsimd.memset(g1_slabs[0][:], 0.0)
            for u in range(10, 20):
                nc.gpsimd.memset(a3[u][32:33, :], 1.0)
            nc.gpsimd.memset(g2_slabs[1][0][:], 0.0)
            nc.gpsimd.memset(g2_slabs[1][1][:], 0.0)
            nc.gpsimd.memset(g1_slabs[1][:], 0.0)

            for s in range(NBLK):
                # ---- transpose x block to feature-major (20, 512) ----
                xt_ps = ps_x.tile([20, 512], f32, tag="xt_ps")
                for i in range(4):
                    nc.tensor.transpose(
                        xt_ps[:, i*128:(i+1)*128],
                        x_sb[:, s*4 + i, :],
                        id_sb[:],
                    )
                xt_sb = xtpool.tile([20, 512], f16, tag="xt")
                copy(xt_sb[:], xt_ps[:])

                # ---- L1: 5 matmuls (K=20, M=128, N=512) ----
                a1 = apool.tile([128, 5, 512], f16, tag="a1")
                for q in range(5):
                    ps = ps_a.tile([128, 512], f32, tag="mm")
                    nc.tensor.matmul(
                        ps[:], w1_sb[:, q*128:(q+1)*128], xt_sb[:]
                    )
                    relu_bias(a1[:, q, :], ps[:], b1_sb[:])

                def l4_assemble(u, t):
                    tg = s*4 + t
                    sg2 = g2_slabs[tg % 2]
                    sg1 = g1_slabs[tg % 2]
                    ps4 = ps_4.tile([128, 146], f32, tag="mm4",
                                    name=f"ps4_{tg}_{u}")
                    nc.tensor.matmul(
                        ps4[:],
                        a3[u][:, t*128:(t+1)*128],
                        w4_sb[:],
                    )
                    # f1|f2 -> f12 slab
                    copy(f12_slab[:, tg, 6*u:6*u+6], ps4[:, 0:6])
                    # g1 band (15 wide at col 5(u-1) mod 100)
                    s1 = (5 * (u - 1)) % 100
                    if s1 + 15 <= 100:
                        copy(sg1[:, u, s1:s1+15], ps4[:, 6:21])
                    else:
                        k = 100 - s1
                        copy(sg1[:, u, s1:100], ps4[:, 6:6+k])
                        copy(sg1[:, u, 0:15-k], ps4[:, 6+k:21])
                    # g2 band (5 rows x 25 at col 5(u-2) mod 100)
                    src = ps4[:, 21:146].rearrange("p (z j) -> p z j", z=5)
                    s2 = (5 * (u - 2)) % 100
                    sg2h = sg2[u // 10]
                    r = 5 * (u % 10)
                    if s2 + 25 <= 100:
                        copy(sg2h[:, r:r+5, s2:s2+25], src)
                    else:
                        k = 100 - s2
                        copy(sg2h[:, r:r+5, s2:100], src[:, :, 0:k])
                        copy(sg2h[:, r:r+5, 0:25-k], src[:, :, k:25])

                def g2_dma(t, h, quarters):
                    tg = s*4 + t
                    sg2 = g2_slabs[tg % 2]
                    dst = g2_d[tg*128:(tg+1)*128, 50*h:50*h+50, :]
                    if quarters:
                        for i in range(2):
                            nc.sync.dma_start(
                                out=dst[:, 25*i:25*(i+1), :],
                                in_=sg2[h][:, 25*i:25*(i+1), :],
                            )
                    else:
                        nc.sync.dma_start(out=dst, in_=sg2[h][:])

                # ---- L2 + L3 per group, with chunk t=0 L4 interleaved ----
                for g in range(10):
                    h, q = g % 2, g // 2
                    ps = ps_a.tile([128, 512], f32, tag="mm")
                    nc.tensor.matmul(
                        ps[:],
                        w2_sb[64*h:64*h+64, :],
                        a1[64*h:64*h+64, q, :],
                    )
                    a2g = a2pool.tile([128, 512], f16, tag="a2")
                    relu_bias(a2g[:], ps[:], b2_sb[:])
                    for ul in range(2):
                        u = 2*g + ul
                        ps3 = ps_3.tile([32, 512], f32, tag="mm3")
                        nc.tensor.matmul(
                            ps3[:],
                            w3_sb[64*ul:64*ul+64, :],
                            a2g[64*ul:64*ul+64, :],
                        )
                        relu_bias(a3[u][0:32, :], ps3[:], b3_sb[:])
                        if INTERLEAVE_T0:
                            l4_assemble(u, 0)
                    if INTERLEAVE_T0 and s == 0 and g == 2:
                        tg0 = s*4
                        nc.sync.dma_start(
                            out=g2_d[tg0*128:(tg0+1)*128, 0:25, :],
                            in_=g2_slabs[tg0 % 2][0][:, 0:25, :],
                        )
                    if INTERLEAVE_T0 and g == 4:
                        if s == 0:
                            tg0 = s*4
                            nc.sync.dma_start(
                                out=g2_d[tg0*128:(tg0+1)*128, 25:50, :],
                                in_=g2_slabs[tg0 % 2][0][:, 25:50, :],
                            )
                        else:
                            g2_dma(0, 0, quarters=False)
                if INTERLEAVE_T0:
                    g2_dma(0, 1, quarters=False)
                    nc.sync.dma_start(
                        out=g1_d[(s*4)*128:(s*4+1)*128], in_=g1_slabs[0][:]
                    )

                # ---- L4 + assembly for remaining chunks ----
                for t in range(1 if INTERLEAVE_T0 else 0, 4):
                    tg = s*4 + t
                    for u in range(20):
                        l4_assemble(u, t)
                    last = (s == NBLK - 1) and (t == 3)
                    g2_dma(t, 0, quarters=last)
                    g2_dma(t, 1, quarters=last)
                    nc.sync.dma_start(
                        out=g1_d[tg*128:(tg+1)*128], in_=g1_slabs[tg % 2][:]
                    )
                nc.sync.dma_start(
                    out=f12_d.rearrange("(t p) u c -> p t (u c)",
                                        p=128)[:, s*4:(s+1)*4, :],
                    in_=f12_slab[:, s*4:(s+1)*4, :],
                )

    nc.compile()
    return nc


def kernel(**inputs):
    global LAST_RESULT
    from concourse.bass_utils import run_bass_kernel_spmd

    x = np.ascontiguousarray(np.asarray(inputs["x"], np.float32))
    assert x.shape == (B, 20)
    consts = _pack_weights(inputs)

    nc = _build_nc()
    in_maps = []
    for c in range(NCORES):
        m = {"x": np.ascontiguousarray(x[c*BC:(c+1)*BC])}
        m.update(consts)
        in_maps.append(m)

    try:
        res = run_bass_kernel_spmd(nc, in_maps, list(range(NCORES)))
    except ModuleNotFoundError:
        # BASS_TRACE in the env routes to an NTFF hook module this image
        # lacks; force tracing off and retry
        os.environ["BASS_NEVER_TRACE"] = "1"
        res = run_bass_kernel_spmd(nc, in_maps, list(range(NCORES)))
    LAST_RESULT = res
    results = res.results

    f12 = np.concatenate([r["f12"] for r in results], axis=0)
    g1 = np.concatenate([r["g1"] for r in results], axis=0)
    g2 = np.concatenate([r["g2"] for r in results], axis=0)

    f1 = np.ascontiguousarray(f12[:, :, 0:1])
    f2 = np.ascontiguousarray(f12[:, :, 1:6].reshape(B, 100, 1))
    return (f1, g1, f2, g2)
